# revision 1
# baseline (speedup 1.0000x reference)
"""Per-patch dynamic conv (nn_DynaMicConv) as a Bass/Tile kernel on 8 TRN2 cores.

Math: for each patch p of a 14x14 grid over a 224x224 image, out[b, :, p] =
W[p] @ patch_pixels[b, p] + bias[p], i.e. 196 independent [64,768] x [768,768]
matmuls. This is DMA-bound: the weight stack is 462 MB and every byte is read
once.

Sharding: patch-parallel. P=196 patches are padded to 200 and split 25 per
core; each core reads only its weight slice, its patch pixels, and writes its
output slice.

Layouts are precomputed on host so every device DMA is a large transfer with
long per-partition contiguous runs (big SDMA descriptors), GROUPS[g] patches
per DMA (per group size s there is one DRAM tensor "ws"/"bs"):
  w<s>  [n_s, 128, s*6*(768+64)]  partition k holds the group's W (rhs chunks
                                  [128,768] per (p,kc), kc-major) followed by
                                  its patch pixels (lhsT chunks [128,64]) --
                                  one DMA delivers a group's W AND x
  b<s>   [n_s, s*768]
  out    [64, PPC*768]            partition b; patch-major columns

Compute per patch: PSUM[64, 768] = sum_kc lhsT_kc.T @ rhs_kc (+ bias via a
ones[1,64] stationary matmul with start=True). Matmul dtype selects the
traffic/accuracy point (MODE): f16 halves DMA bytes vs f32r at ~2x its error.
PSUM -> SBUF copy (cast to the output dtype) on DVE; outputs stage in per-
segment SBUF tiles and store in STORE_CUTS chunks so stores never block the
DVE/PE pipeline. Measured (8 axon TRN2 cores, max over cores): ~101.5us best,
~105-118us typical under neighbor contention; rel err 3.6e-4.
"""

import numpy as np

import concourse.bacc as bacc
import concourse.mybir as mybir
import concourse.tile as tile
from concourse.bass_utils import run_bass_kernel_spmd

B, CIN, IMG, PS, G = 64, 3, 224, 16, 14
P = G * G                 # 196 patches
COUT = 768
K = CIN * PS * PS         # 768 contraction
KCH = K // 128            # 6 k-chunks
NCORES = 8
PPC = (P + NCORES - 1) // NCORES   # 25 patches per core (padded)
PPAD = PPC * NCORES                # 200
# Patches per W/x/bias DMA. Big groups amortize the ~2us per-dma_start
# completion receipt that serializes each HWDGE ring and keep SDMA
# descriptors large (46KB/partition-row); the taper at the end shortens the
# PE tail after the final W byte lands and keeps the PE fed (warm) to the
# finish. W streams on the SP ring; x/bias/output stores ride the ACT ring.
GROUPS = [5, 5, 5, 4, 3, 2, 1]
assert sum(GROUPS) == PPC
# output store split points (patch indices)
STORE_CUTS = [0, 5, 10, 15, 20, 23, PPC]

F32 = mybir.dt.float32

# matmul input dtype: 'f16' (half DMA traffic, ~3e-4 rel err),
# 'f32r' (full fp32 traffic, ~1.5e-4), 'bf16' (half traffic, ~2e-3)
MODE = "f16"
_DTYPES = {
    "f32r": (mybir.dt.float32r, np.float32),
    "f16": (mybir.dt.float16, np.float16),
    "bf16": (mybir.dt.bfloat16, None),  # np dtype resolved lazily via ml_dtypes
}

# store outputs as fp16 (halves store traffic; adds ~1.4e-4 rms rounding)
OUT_F16 = True

# buffer depths (in DMA groups)
WBUFS = 3
XBUFS = 3

# test.py hooks: set TRACE=True before calling kernel() to profile; the
# BassKernelResults of the last run lands in LAST_RESULT.
TRACE = False
TRACE_CORES = [0]
LAST_RESULT = None

_CACHE = {}


def _np_dtype(mode):
    mdt, ndt = _DTYPES[mode]
    if ndt is None:
        import ml_dtypes
        ndt = ml_dtypes.bfloat16
    return mdt, ndt


def _build(mode):
    mdt, _ = _np_dtype(mode)
    odt = mybir.dt.float16 if OUT_F16 else F32
    nc = bacc.Bacc("TRN2", target_bir_lowering=False, debug=False)
    # one DRAM tensor per distinct group size s: [count_s, 128, s*...]
    sizes = sorted(set(GROUPS))
    cnt = {s: GROUPS.count(s) for s in sizes}
    # W and x for a group ride ONE DMA: [.., 128, s*KCH*COUT | s*KCH*B]
    w_d = {s: nc.dram_tensor(f"w{s}", [cnt[s], 128, s * KCH * (COUT + B)], mdt,
                             kind="ExternalInput") for s in sizes}
    b_d = {s: nc.dram_tensor(f"b{s}", [cnt[s], s * COUT], mdt,
                             kind="ExternalInput") for s in sizes}
    ones_d = nc.dram_tensor("ones", [1, B], mdt, kind="ExternalInput")
    o_d = nc.dram_tensor("out", [B, PPC * COUT], odt, kind="ExternalOutput")

    gmax = max(GROUPS)
    with tile.TileContext(nc) as tc:
        with (
            tc.tile_pool(name="const", bufs=1) as cpool,
            tc.tile_pool(name="wp", bufs=WBUFS) as wpool,
            tc.tile_pool(name="bp", bufs=XBUFS) as bpool,
            tc.tile_pool(name="op", bufs=3) as opool,
            tc.tile_pool(name="ps", bufs=3, space="PSUM") as pspool,
        ):
            ones = cpool.tile([1, B], mdt)
            nc.scalar.dma_start(ones[:], ones_d[:])

            sidx = {s: 0 for s in sizes}
            poff = 0
            seg = 0
            oseg = None
            for gi, GPS in enumerate(GROUPS):
                j = sidx[GPS]; sidx[GPS] += 1
                wt = wpool.tile([128, gmax * KCH * (COUT + B)], mdt, tag="w")
                nc.sync.dma_start(wt[:, : GPS * KCH * (COUT + B)], w_d[GPS][j])
                bt = bpool.tile([1, gmax * COUT], mdt, tag="b")
                nc.scalar.dma_start(bt[:, : GPS * COUT], b_d[GPS][j])

                for i in range(GPS):
                    ps1 = pspool.tile([B, 512], F32, tag="ps1", bufs=4)
                    ps2 = pspool.tile([B, 256], F32, tag="ps2")
                    boff = i * COUT
                    nc.tensor.matmul(ps1[:], ones[:], bt[:, boff: boff + 512],
                                     start=True, stop=False)
                    nc.tensor.matmul(ps2[:], ones[:], bt[:, boff + 512: boff + COUT],
                                     start=True, stop=False)
                    xbase = GPS * KCH * COUT
                    for kc in range(KCH):
                        xoff = xbase + (i * KCH + kc) * B
                        woff = (i * KCH + kc) * COUT
                        lhs = wt[:, xoff: xoff + B]
                        last = kc == KCH - 1
                        nc.tensor.matmul(ps1[:], lhs,
                                         wt[:, woff: woff + 512],
                                         start=False, stop=last)
                        nc.tensor.matmul(ps2[:], lhs,
                                         wt[:, woff + 512: woff + COUT],
                                         start=False, stop=last)

                    p = poff + i
                    if p == STORE_CUTS[seg]:
                        nseg = STORE_CUTS[seg + 1] - STORE_CUTS[seg]
                        oseg = opool.tile([B, nseg * COUT], odt, tag="o",
                                          name=f"oseg{seg}")
                    coff = (p - STORE_CUTS[seg]) * COUT
                    nc.vector.tensor_copy(oseg[:, coff: coff + 512], ps1[:])
                    nc.vector.tensor_copy(oseg[:, coff + 512: coff + COUT], ps2[:])
                    if p + 1 == STORE_CUTS[seg + 1]:
                        nc.scalar.dma_start(
                            o_d[:, STORE_CUTS[seg] * COUT: STORE_CUTS[seg + 1] * COUT],
                            oseg[:])
                        seg += 1
                poff += GPS
    nc.compile()
    return nc


def _prep(x, W, b, mode):
    _, ndt = _np_dtype(mode)
    # patch pixels, k-transposed: xp[p, k, b] with k = c*256 + r*16 + s
    xp = (x.reshape(B, CIN, G, PS, G, PS)
           .transpose(2, 4, 1, 3, 5, 0)
           .reshape(P, K, B))
    # -> [P, 128(kpart), KCH, B]
    xr = np.zeros((PPAD, 128, KCH, B), dtype=ndt)
    xr[:P] = xp.reshape(P, KCH, 128, B).transpose(0, 2, 1, 3).astype(ndt)

    # weights: w[p, kpart, kc*COUT + o] = W[p, o, kc*128 + kpart]
    wr = np.zeros((PPAD, 128, KCH * COUT), dtype=ndt)
    wr[:P] = (W.reshape(P, COUT, KCH, 128)
               .transpose(0, 3, 2, 1)
               .reshape(P, 128, KCH * COUT).astype(ndt))

    br = np.zeros((PPAD, COUT), dtype=ndt)
    br[:P] = b.astype(ndt)
    onesv = np.ones((1, B), dtype=ndt)

    sizes = sorted(set(GROUPS))
    in_maps = []
    for c in range(NCORES):
        base = c * PPC
        m = {"ones": onesv}
        packs = {s: ([], []) for s in sizes}
        poff = 0
        for gs in GROUPS:
            pl = slice(base + poff, base + poff + gs)
            # [gs, 128, cols] -> [128, gs*cols], W block then x block
            wg = wr[pl].transpose(1, 0, 2).reshape(128, gs * KCH * COUT)
            xg = (xr[pl].reshape(gs, 128, KCH * B)
                  .transpose(1, 0, 2).reshape(128, gs * KCH * B))
            packs[gs][0].append(np.concatenate([wg, xg], axis=1))
            packs[gs][1].append(br[pl].reshape(gs * COUT))
            poff += gs
        for s in sizes:
            m[f"w{s}"] = np.ascontiguousarray(np.stack(packs[s][0]))
            m[f"b{s}"] = np.ascontiguousarray(np.stack(packs[s][1]))
        in_maps.append(m)
    return in_maps


def kernel(x, W, b):
    global LAST_RESULT
    x = np.ascontiguousarray(np.asarray(x, dtype=np.float32))
    W = np.ascontiguousarray(np.asarray(W, dtype=np.float32))
    b = np.ascontiguousarray(np.asarray(b, dtype=np.float32))
    in_maps = _prep(x, W, b, MODE)
    key = ("nc", MODE, OUT_F16, tuple(GROUPS), WBUFS, XBUFS)
    if key not in _CACHE:
        _CACHE[key] = _build(MODE)
    res = run_bass_kernel_spmd(
        _CACHE[key], in_maps, core_ids=list(range(NCORES)),
        trace=TRACE, trace_cores=TRACE_CORES,
    )
    LAST_RESULT = res
    # assemble: padded patch order is [c*PPC + i]; patches 196..199 are pad
    obig = np.concatenate(
        [res.results[c]["out"].reshape(B, PPC, COUT) for c in range(NCORES)],
        axis=1)                                   # [B, PPAD, COUT]
    out = obig[:, :P].astype(np.float32)          # [B, P, COUT]
    return np.ascontiguousarray(out.transpose(0, 2, 1)).reshape(B, COUT, G, G)



# revision 4
# speedup vs baseline: 1.4057x; 1.4057x over previous
"""Per-patch dynamic conv (nn_DynaMicConv) as a Bass/Tile kernel on 8 TRN2 cores.

Math: for each patch p of a 14x14 grid over a 224x224 image, out[b, :, p] =
W[p] @ patch_pixels[b, p] + bias[p], i.e. 196 independent [64,768] x [768,768]
matmuls. DMA-bound: the 462 MB (f32) weight stack is read exactly once.

v2 vs the f16 baseline (101.8us):
  * W rides as fp8 E3M4 (TRN float8e3, 4 mantissa bits): half the W bytes of
    f16 at ~1.32e-2 rel err (measured host-side; gate is 2e-2). A global
    power-of-2 scale (W*128 fits in e3m4's +-15.5 range) is folded into x
    (x/128, exact in f16), so PSUM results come out in true units; x, bias,
    and the output stay f16. Per-core traffic drops 34.4MB -> 19.7MB.
  * At that traffic the PE becomes co-critical (moving operand streams 1
    col/cycle regardless of dtype), so patches are processed in PAIRS via
    column tiling: patch 2j's matmuls land in PE columns 0-63 (PSUM
    partitions 0-63), patch 2j+1's in columns 64-127 -- concurrent in the
    array, halving PE wall time (~28us busy).
  * x is loaded resident up front (one DMA); W groups alternate between the
    two HWDGE rings (sync/scalar) so per-DMA completion receipts overlap;
    output stores ride SWDGE (gpsimd) off the load rings.

Sharding: patch-parallel, P=196 padded to 200, 25 per core (12 pairs + 1).
Output DRAM per core is [128, 13*768] f16: col block j holds pair j (rows
0:64 = patch 2j, rows 64:128 = patch 2j+1); block 12 rows 0:64 = patch 24.
"""

import numpy as np

import concourse.bacc as bacc
import concourse.mybir as mybir
import concourse.tile as tile
from concourse.bass_utils import run_bass_kernel_spmd

B, CIN, IMG, PS, G = 64, 3, 224, 16, 14
P = G * G                 # 196 patches
COUT = 768
K = CIN * PS * PS         # 768 contraction
KCH = K // 128            # 6 k-chunks
NCORES = 8
PPC = (P + NCORES - 1) // NCORES   # 25 patches per core (padded)
PPAD = PPC * NCORES                # 200
NPAIR = PPC // 2                   # 12 full pairs; patch 24 runs alone
NBLK = NPAIR + 1                   # output col blocks

# patches per W DMA (even sizes so groups hold whole pairs; taper at the end
# shortens the post-last-byte compute tail). Even-indexed groups ride the
# sync (SP) HWDGE ring, odd-indexed the scalar (ACT) ring.
GROUPS = [4, 6, 6, 6, 2, 1]
assert sum(GROUPS) == PPC
# output store cuts, in pair blocks (final single patch stored separately)
STORE_CUTS = [0, 2, 5, 8, 11, 12]

F32 = mybir.dt.float32
F16 = mybir.dt.float16
F8 = mybir.dt.float8e3   # TRN E3M4

WSCALE = 128.0           # power of 2: folded into x exactly
E3M4_MAX = 15.5

WBUFS = 3

# test.py hooks
TRACE = False
TRACE_CORES = [0]
LAST_RESULT = None

_CACHE = {}

WB = KCH * COUT          # W bytes per patch per partition (fp8)


def _build():
    nc = bacc.Bacc("TRN2", target_bir_lowering=False, debug=False)
    sizes = sorted(set(GROUPS))
    cnt = {s: GROUPS.count(s) for s in sizes}
    w_d = {s: nc.dram_tensor(f"w{s}", [cnt[s], 128, s * WB], F8,
                             kind="ExternalInput") for s in sizes}
    x_d = nc.dram_tensor("x", [128, PPC * KCH * B], F16, kind="ExternalInput")
    bo_d = nc.dram_tensor("bo", [1, B + PPC * COUT], F16, kind="ExternalInput")
    o_d = nc.dram_tensor("out", [128, NBLK * COUT], F16, kind="ExternalOutput")

    gmax = max(GROUPS)
    with tile.TileContext(nc) as tc:
        with (
            tc.tile_pool(name="const", bufs=1) as cpool,
            tc.tile_pool(name="wp", bufs=WBUFS) as wpool,
            tc.tile_pool(name="op", bufs=3) as opool,
            tc.tile_pool(name="ps", bufs=3, space="PSUM") as pspool,
        ):
            bo = cpool.tile([1, B + PPC * COUT], F16)
            nc.scalar.dma_start(bo[:], bo_d[:])
            xt = cpool.tile([128, PPC * KCH * B], F16)
            nc.scalar.dma_start(xt[:], x_d[:])
            ones = bo[:, 0:B]

            def bias(p):  # [1, 768] slice for patch p
                return bo[:, B + p * COUT: B + (p + 1) * COUT]

            sidx = {s: 0 for s in sizes}
            poff = 0          # first patch of current group
            seg = 0           # store segment index (in pair blocks)
            oseg = None
            for gi, GPS in enumerate(GROUPS):
                j = sidx[GPS]; sidx[GPS] += 1
                wt = wpool.tile([128, gmax * WB], F8, tag="w")
                eng = nc.sync if gi % 2 == 0 else nc.scalar
                eng.dma_start(wt[:, : GPS * WB], w_d[GPS][j])

                for i in range(0, GPS, 2):
                    p0 = poff + i
                    single = p0 == PPC - 1
                    rows = 64 if single else 128
                    ps1 = pspool.tile([128, 512], F32, tag="ps1", bufs=4)
                    ps2 = pspool.tile([128, 256], F32, tag="ps2")
                    b0 = bias(p0)
                    nc.tensor.matmul(ps1[0:64, :], ones, b0[:, 0:512],
                                     start=True, stop=False)
                    nc.tensor.matmul(ps2[0:64, :], ones, b0[:, 512:COUT],
                                     start=True, stop=False)
                    if not single:
                        # skip_group_check: CoreSim's PSUM accumulation-group
                        # tracker keys on the zero region without the base
                        # partition, so the 64:128 col-tile half falsely
                        # collides with the 0:64 half. HW has_written bits
                        # are per element; the halves are disjoint.
                        b1 = bias(p0 + 1)
                        nc.tensor.matmul(ps1[64:128, :], ones, b1[:, 0:512],
                                         start=True, stop=False,
                                         skip_group_check=True)
                        nc.tensor.matmul(ps2[64:128, :], ones, b1[:, 512:COUT],
                                         start=True, stop=False,
                                         skip_group_check=True)
                    for kc in range(KCH):
                        last = kc == KCH - 1
                        for h in range(1 if single else 2):
                            p = p0 + h
                            lo, hi = 64 * h, 64 * h + 64
                            xs = xt[:, (p * KCH + kc) * B: (p * KCH + kc + 1) * B]
                            woff = ((i + h) * KCH + kc) * COUT
                            nc.tensor.matmul(ps1[lo:hi, :], xs,
                                             wt[:, woff: woff + 512],
                                             start=False, stop=last,
                                             skip_group_check=h == 1)
                            nc.tensor.matmul(ps2[lo:hi, :], xs,
                                             wt[:, woff + 512: woff + COUT],
                                             start=False, stop=last,
                                             skip_group_check=h == 1)

                    blk = p0 // 2
                    if single:
                        oseg = opool.tile([64, COUT], F16, tag="olast")
                        nc.vector.tensor_copy(oseg[:, 0:512], ps1[0:64, :])
                        nc.vector.tensor_copy(oseg[:, 512:COUT], ps2[0:64, :])
                        nc.gpsimd.dma_start(
                            o_d[0:64, NPAIR * COUT: NBLK * COUT], oseg[:])
                    else:
                        if blk == STORE_CUTS[seg]:
                            nseg = STORE_CUTS[seg + 1] - STORE_CUTS[seg]
                            oseg = opool.tile([128, nseg * COUT], F16, tag="o",
                                              name=f"oseg{seg}")
                        coff = (blk - STORE_CUTS[seg]) * COUT
                        nc.vector.tensor_copy(oseg[:, coff: coff + 512], ps1[:])
                        nc.vector.tensor_copy(oseg[:, coff + 512: coff + COUT],
                                              ps2[:])
                        if blk + 1 == STORE_CUTS[seg + 1]:
                            nc.gpsimd.dma_start(
                                o_d[:, STORE_CUTS[seg] * COUT:
                                     STORE_CUTS[seg + 1] * COUT], oseg[:])
                            seg += 1
                poff += GPS
    nc.compile()
    return nc


def _prep(x, W, b):
    import ml_dtypes
    f8 = ml_dtypes.float8_e3m4
    scale = WSCALE
    wmax = float(np.abs(W).max())
    while wmax * scale > E3M4_MAX:
        scale /= 2.0
    # patch pixels, k-transposed: xp[p, k, b], k = c*256 + r*16 + s
    xp = (x.reshape(B, CIN, G, PS, G, PS)
           .transpose(2, 4, 1, 3, 5, 0)
           .reshape(P, K, B)) * (1.0 / scale)
    # resident x: [128(kpart), p, kc, b]
    xr = np.zeros((128, PPAD, KCH, B), dtype=np.float16)
    xr[:, :P] = (xp.reshape(P, KCH, 128, B)
                 .transpose(2, 0, 1, 3).astype(np.float16))
    xr = xr.reshape(128, PPAD * KCH * B)

    # weights: wr[p, kpart, kc*COUT + o] = W[p, o, kc*128 + kpart] * scale
    wr = np.zeros((PPAD, 128, WB), dtype=f8)
    wr[:P] = np.clip(
        (W.reshape(P, COUT, KCH, 128) * scale)
        .transpose(0, 3, 2, 1).reshape(P, 128, WB),
        -E3M4_MAX, E3M4_MAX).astype(f8)

    br = np.zeros((PPAD, COUT), dtype=np.float16)
    br[:P] = b.astype(np.float16)

    sizes = sorted(set(GROUPS))
    in_maps = []
    for c in range(NCORES):
        base = c * PPC
        bo = np.empty((1, B + PPC * COUT), dtype=np.float16)
        bo[0, :B] = 1.0
        bo[0, B:] = br[base: base + PPC].reshape(-1)
        m = {
            "bo": bo,
            "x": np.ascontiguousarray(
                xr[:, base * KCH * B: (base + PPC) * KCH * B]),
        }
        packs = {s: [] for s in sizes}
        poff = 0
        for gs in GROUPS:
            pl = slice(base + poff, base + poff + gs)
            packs[gs].append(
                wr[pl].transpose(1, 0, 2).reshape(128, gs * WB))
            poff += gs
        for s in sizes:
            m[f"w{s}"] = np.ascontiguousarray(np.stack(packs[s]))
        in_maps.append(m)
    return in_maps


def kernel(x, W, b):
    global LAST_RESULT
    x = np.ascontiguousarray(np.asarray(x, dtype=np.float32))
    W = np.ascontiguousarray(np.asarray(W, dtype=np.float32))
    b = np.ascontiguousarray(np.asarray(b, dtype=np.float32))
    in_maps = _prep(x, W, b)
    key = ("nc", tuple(GROUPS), WBUFS)
    if key not in _CACHE:
        _CACHE[key] = _build()
    res = run_bass_kernel_spmd(
        _CACHE[key], in_maps, core_ids=list(range(NCORES)),
        trace=TRACE, trace_cores=TRACE_CORES,
    )
    LAST_RESULT = res
    # assemble: core c block j rows 0:64 -> patch c*PPC+2j, rows 64:128 ->
    # patch c*PPC+2j+1; block NPAIR rows 0:64 -> patch c*PPC+24
    out = np.empty((B, PPAD, COUT), dtype=np.float32)
    for c in range(NCORES):
        ob = np.asarray(res.results[c]["out"]).reshape(128, NBLK, COUT)
        base = c * PPC
        out[:, base: base + PPC - 1: 2] = ob[0:64, :NPAIR].astype(np.float32)
        out[:, base + 1: base + PPC: 2] = ob[64:128, :NPAIR].astype(np.float32)
        out[:, base + PPC - 1] = ob[0:64, NPAIR].astype(np.float32)
    out = out[:, :P]
    return np.ascontiguousarray(out.transpose(0, 2, 1)).reshape(B, COUT, G, G)
